# revision 16
# baseline (speedup 1.0000x reference)
"""Trainium2 Bass kernel for an autoregressive transformer sampler.

Model: 32-step incremental decode, 4 transformer layers, d_model=128,
4 heads x 32 head-dim, ffn 512, vocab-4 head with electron-budget
validity masking and Gumbel-max categorical sampling.

Sharding: pure data parallel. 1024 samples -> 8 cores x 128 samples.
On-core layout: batch-major activations (sample on the SBUF partition
axis), feature-major weights; all matmuls on the PE via transposes,
attention elementwise streams split between GpSimd (muls) and Vector
(segmented reduces), transcendentals on the Scalar engine.
"""

import os
import numpy as np

N_ORB = 32
D = 128
H = 4
HD = 32
L = 4
DFF = 512
NA, NB = 16, 16
BOS = 4
B = 128          # samples per core
NCORES = 8
NEG = -1e9
SCALE = 1.0 / np.sqrt(HD)

_PROG_CACHE = {}


def _build_program():
    from contextlib import ExitStack
    import concourse.bass as bass
    import concourse.tile as tile
    from concourse import bacc, mybir

    F = mybir.ActivationFunctionType
    A = mybir.AluOpType
    f32 = mybir.dt.float32

    nc = bacc.Bacc("TRN2", target_bir_lowering=False, debug=False)

    din = {}
    def dram_in(name, shape):
        din[name] = nc.dram_tensor(name, list(shape), f32, kind="ExternalInput")
        return din[name]

    dram_in("wqkv", (L, D, 3 * D))
    dram_in("bqkv", (L, D, 3))
    dram_in("wout", (L, D, D))
    dram_in("bout", (L, D, 1))
    dram_in("wff1", (L, D, DFF))
    dram_in("bff1", (L, D, 4))
    dram_in("wff2", (L, 4, D, D))
    dram_in("bff2", (L, D, 1))
    dram_in("whead", (D, 4))
    dram_in("bheadc", (4, 1))
    dram_in("emb4", (4, D))
    dram_in("x0r", (B, D))
    dram_in("posT", (D, N_ORB))
    dram_in("gum", (B, 4 * N_ORB))
    dram_in("cA", (B, 4))
    dram_in("cB", (B, 4))
    dram_in("iota4", (B, 4))
    dram_in("ident", (D, D))

    cfg_out = nc.dram_tensor("cfg", [B, 2 * N_ORB], f32, kind="ExternalOutput")
    lgp_out = nc.dram_tensor("lgp", [B, 1], f32, kind="ExternalOutput")

    with tile.TileContext(nc) as tc, ExitStack() as ctx:
        S = ctx.enter_context(tc.tile_pool(name="singles", bufs=1))
        W = ctx.enter_context(tc.tile_pool(name="work", bufs=2))
        P = ctx.enter_context(tc.tile_pool(name="psum", bufs=3, space="PSUM"))
        PF = ctx.enter_context(tc.tile_pool(name="psumff", bufs=2, space="PSUM"))

        # ---- load constants / weights ----
        def load(name, shape, src_ap):
            t = S.tile(list(shape), f32, tag=name, name=name)
            nc.sync.dma_start(t[:], src_ap)
            return t

        wqkv = [load(f"wqkv{l}", (D, 3 * D), din["wqkv"].ap()[l]) for l in range(L)]
        bqkv = [load(f"bqkv{l}", (D, 3), din["bqkv"].ap()[l]) for l in range(L)]
        wout = [load(f"wout{l}", (D, D), din["wout"].ap()[l]) for l in range(L)]
        bout = [load(f"bout{l}", (D, 1), din["bout"].ap()[l]) for l in range(L)]
        wff1 = [load(f"wff1{l}", (D, DFF), din["wff1"].ap()[l]) for l in range(L)]
        bff1 = [load(f"bff1{l}", (D, 4), din["bff1"].ap()[l]) for l in range(L)]
        wff2 = [[load(f"wff2{l}_{c}", (D, D), din["wff2"].ap()[l, c]) for c in range(4)]
                for l in range(L)]
        bff2 = [load(f"bff2{l}", (D, 1), din["bff2"].ap()[l]) for l in range(L)]
        whead = load("whead", (D, 4), din["whead"].ap())
        bheadc = load("bheadc", (4, 1), din["bheadc"].ap())
        emb4 = load("emb4", (4, D), din["emb4"].ap())
        posT = load("posT", (D, N_ORB), din["posT"].ap())
        gum = load("gum", (B, 4 * N_ORB), din["gum"].ap())
        cA = load("cA", (B, 4), din["cA"].ap())
        cB = load("cB", (B, 4), din["cB"].ap())
        iota4 = load("iota4", (B, 4), din["iota4"].ap())
        ident = load("ident", (D, D), din["ident"].ap())

        # ---- persistent state ----
        Kc = [S.tile([B, H * N_ORB * HD], f32, tag=f"Kc{l}", name=f"Kc{l}") for l in range(L)]
        Vc = [S.tile([B, H * HD * N_ORB], f32, tag=f"Vc{l}", name=f"Vc{l}") for l in range(L)]
        tmp1 = S.tile([B, H * N_ORB * HD], f32, tag="tmp1")
        tmp2 = tmp1
        cfg = S.tile([B, 2 * N_ORB], f32, tag="cfg")
        lgp = S.tile([B, 1], f32, tag="lgp")
        a_rem = S.tile([B, 1], f32, tag="a_rem")
        b_rem = S.tile([B, 1], f32, tag="b_rem")
        x = S.tile([B, D], f32, tag="x")

        nc.vector.memset(lgp[:], 0.0)
        nc.vector.memset(a_rem[:], float(NA))
        nc.vector.memset(b_rem[:], float(NB))
        nc.sync.dma_start(x[:], din["x0r"].ap())

        def transpose_pe(src_sb, p, f):
            """src [p, f] sbuf -> psum [f, p]"""
            t = P.tile([128, 128], f32, tag="ps", name="tps")
            nc.tensor.matmul(t[:f, :p], src_sb, ident[:p, :p], is_transpose=True)
            return t

        def layernorm_stats(xin):
            """xin [B, D] sbuf -> rstd [B,1] (1 Newton step), mean [B,1]"""
            st6 = W.tile([B, 6], f32, tag="st6")
            nc.vector.bn_stats(st6[:], xin[:])
            mv = W.tile([B, 2], f32, tag="mv")
            nc.vector.bn_aggr(mv[:], st6[:])
            mean = mv[:, 0:1]
            vp = W.tile([B, 1], f32, tag="vp")
            nc.vector.tensor_scalar(vp[:], mv[:, 1:2], 1e-5, None, op0=A.add)
            s = W.tile([B, 1], f32, tag="lns")
            nc.scalar.activation(s[:], vp[:], F.Sqrt)
            r0 = W.tile([B, 1], f32, tag="lnr0")
            nc.vector.reciprocal(r0[:], s[:])
            r2 = W.tile([B, 1], f32, tag="lnr2")
            nc.scalar.activation(r2[:], r0[:], F.Square)
            w1 = W.tile([B, 1], f32, tag="lnw1")
            nc.vector.tensor_scalar(w1[:], r2[:], vp[:], -0.5, op0=A.mult, op1=A.mult)
            rstd = W.tile([B, 1], f32, tag="lnrstd")
            nc.vector.scalar_tensor_tensor(rstd[:], w1[:], 1.5, r0[:],
                                           op0=A.add, op1=A.mult)
            return mean, rstd

        def normalize_T(xin):
            """ln(xin) (w/b folded into weights) -> sbuf [D, B] feature-major"""
            mean, rstd = layernorm_stats(xin)
            xn = W.tile([B, D], f32, tag="xn", name="xn")
            nc.vector.tensor_scalar(xn[:], xin[:], mean, rstd, op0=A.subtract,
                                    op1=A.mult)
            xnT_ps = transpose_pe(xn[:], B, D)
            xnT = W.tile([D, B], f32, tag="xnT", name="xnT")
            nc.scalar.copy(xnT[:], xnT_ps[:D, :B])
            return xnT

        for t in range(N_ORB):
            for l in range(L):
                # ---- LN1 + qkv ----
                h1T = normalize_T(x[:])
                qkvT_ps = P.tile([D, 3 * B], f32, tag="ps")
                for c in range(3):
                    nc.tensor.matmul(qkvT_ps[:, c * B:(c + 1) * B],
                                     wqkv[l][:, c * D:(c + 1) * D], h1T[:])
                # bias per m-partition, move to sbuf feature-major
                qkvT = W.tile([D, 3 * B], f32, tag="qkvT")
                for c in range(3):
                    nc.vector.tensor_scalar_add(
                        qkvT[:, c * B:(c + 1) * B],
                        qkvT_ps[:, c * B:(c + 1) * B],
                        bqkv[l][:, c:c + 1])
                # transpose each to batch-major
                q_bm = W.tile([B, D], f32, tag="q_bm")
                qps = transpose_pe(qkvT[:, 0 * B:1 * B], D, B)
                nc.scalar.copy(q_bm[:], qps[:B, :D])
                kps = transpose_pe(qkvT[:, 1 * B:2 * B], D, B)
                Kc4 = Kc[l][:].rearrange("p (h t d) -> p h t d", h=H, t=N_ORB)
                nc.vector.tensor_copy(
                    Kc4[:, :, t, :], kps[:B, :D].rearrange("p (h d) -> p h d", h=H))
                vps = transpose_pe(qkvT[:, 2 * B:3 * B], D, B)
                Vc4 = Vc[l][:].rearrange("p (h d t) -> p h d t", h=H, d=HD)
                nc.vector.tensor_copy(
                    Vc4[:, :, :, t], vps[:B, :D].rearrange("p (h d) -> p h d", h=H))

                # ---- attention (per-head: stt/reduce are limited to 3D) ----
                T1 = t + 1
                tmp1v = tmp1[:].rearrange("p (h t d) -> p h t d", h=H, t=N_ORB)
                tmp2v = tmp2[:].rearrange("p (h d t) -> p h d t", h=H, d=HD)
                qh = q_bm[:].rearrange("p (h d) -> p h d", h=H)
                sc = W.tile([B, H * N_ORB], f32, tag="sc")
                sc3 = sc[:].rearrange("p (h t) -> p h t", h=H)[:, :, :T1]
                ee = W.tile([B, H * N_ORB], f32, tag="ee")
                ee3 = ee[:].rearrange("p (h t) -> p h t", h=H)[:, :, :T1]
                for h in range(H):
                    t1v = tmp1v[:, h, :T1, :]
                    qb = qh[:, h, :].unsqueeze(1).broadcast_to([B, T1, HD])
                    nc.gpsimd.tensor_tensor(t1v, Kc4[:, h, :T1, :], qb,
                                            op=A.mult)
                    nc.vector.tensor_reduce(sc3[:, h, :], t1v,
                                            axis=mybir.AxisListType.X, op=A.add)
                nc.scalar.activation(ee3, sc3, F.Exp, scale=float(SCALE))
                se = W.tile([B, H], f32, tag="se")
                nc.vector.tensor_reduce(se[:].unsqueeze(2), ee3,
                                        axis=mybir.AxisListType.X, op=A.add)
                rse = W.tile([B, H], f32, tag="rse")
                nc.vector.reciprocal(rse[:], se[:])
                att = W.tile([B, D], f32, tag="att")
                att3 = att[:].rearrange("p (h d) -> p h d", h=H)
                for h in range(H):
                    t2v = tmp2v[:, h, :, :T1]
                    eb = ee3[:, h, :].unsqueeze(1).broadcast_to([B, HD, T1])
                    nc.gpsimd.tensor_tensor(t2v, Vc4[:, h, :, :T1], eb,
                                            op=A.mult)
                    nc.vector.tensor_reduce(att3[:, h, :], t2v,
                                            axis=mybir.AxisListType.X, op=A.add)
                attn = W.tile([B, D], f32, tag="attn")
                rseb = rse[:].unsqueeze(2).broadcast_to([B, H, HD])
                nc.vector.tensor_tensor(attn[:].rearrange("p (h d) -> p h d", h=H),
                                        att3, rseb, op=A.mult)
                # ---- out proj + residual ----
                attT_ps = transpose_pe(attn[:], B, D)
                attT = W.tile([D, B], f32, tag="attT")
                nc.scalar.copy(attT[:], attT_ps[:D, :B])
                oT_ps = P.tile([D, B], f32, tag="ps")
                nc.tensor.matmul(oT_ps[:], wout[l][:], attT[:])
                oT = W.tile([D, B], f32, tag="oT")
                nc.vector.tensor_scalar_add(oT[:], oT_ps[:], bout[l][:])
                o_ps = transpose_pe(oT[:], D, B)
                x2 = W.tile([B, D], f32, tag="x2")
                nc.vector.tensor_tensor(x2[:], x[:], o_ps[:B, :D], op=A.add)

                # ---- LN2 + FFN ----
                h2T = normalize_T(x2[:])
                g1c = []
                for c in range(4):
                    f1_ps = PF.tile([D, B], f32, tag="f1ps")
                    nc.tensor.matmul(f1_ps[:], wff1[l][:, c * D:(c + 1) * D], h2T[:])
                    g1 = W.tile([D, B], f32, tag=f"g1_{c}", name=f"g1_{c}")
                    nc.scalar.activation(g1[:], f1_ps[:],
                                         F.Gelu, bias=bff1[l][:, c:c + 1])
                    g1c.append(g1)
                f2_ps = P.tile([D, B], f32, tag="f2ps", bufs=1, name="f2_ps")
                for c in range(4):
                    nc.tensor.matmul(f2_ps[:], wff2[l][c][:], g1c[c][:],
                                     start=(c == 0), stop=(c == 3))
                f2T = W.tile([D, B], f32, tag="f2T")
                nc.vector.tensor_scalar_add(f2T[:], f2_ps[:], bff2[l][:])
                f2b_ps = transpose_pe(f2T[:], D, B)
                xnew = S.tile([B, D], f32, tag="x")
                nc.vector.tensor_tensor(xnew[:], x2[:], f2b_ps[:B, :D], op=A.add)
                x = xnew

            # ---- head ----
            xfT = normalize_T(x[:])
            lg_ps = P.tile([4, B], f32, tag="ps")
            nc.tensor.matmul(lg_ps[:], whead[:], xfT[:])
            lgT = W.tile([4, B], f32, tag="lgT")
            nc.scalar.activation(lgT[:], lg_ps[:], F.Identity, bias=bheadc[:])
            lg_ps2 = transpose_pe(lgT[:], 4, B)
            # ---- validity mask ----
            after = float(N_ORB - 1 - t)
            arm = W.tile([B, 1], f32, tag="arm")
            nc.vector.tensor_scalar_add(arm[:], a_rem[:], -after)
            brm = W.tile([B, 1], f32, tag="brm")
            nc.vector.tensor_scalar_add(brm[:], b_rem[:], -after)
            va = W.tile([B, 4], f32, tag="va")
            vb = W.tile([B, 4], f32, tag="vb")
            va2 = W.tile([B, 4], f32, tag="va2")
            nc.vector.tensor_scalar(va[:], cA[:], a_rem[:], None, op0=A.is_le)
            nc.vector.tensor_scalar(va2[:], cA[:], arm[:], None, op0=A.is_ge)
            nc.vector.tensor_tensor(va[:], va[:], va2[:], op=A.mult)
            vb2 = W.tile([B, 4], f32, tag="vb2")
            nc.vector.tensor_scalar(vb[:], cB[:], b_rem[:], None, op0=A.is_le)
            nc.vector.tensor_scalar(vb2[:], cB[:], brm[:], None, op0=A.is_ge)
            nc.vector.tensor_tensor(vb[:], vb[:], vb2[:], op=A.mult)
            valid = W.tile([B, 4], f32, tag="valid")
            nc.vector.tensor_tensor(valid[:], va[:], vb[:], op=A.mult)
            vm1 = W.tile([B, 4], f32, tag="vm1")
            nc.vector.tensor_scalar(vm1[:], valid[:], 1.0, None, op0=A.subtract)
            ml = W.tile([B, 4], f32, tag="ml")
            nc.vector.scalar_tensor_tensor(ml[:], vm1[:], 1e9, lg_ps2[:B, :4],
                                           op0=A.mult, op1=A.add)
            # ---- gumbel argmax ----
            m = W.tile([B, 4], f32, tag="m")
            nc.vector.tensor_tensor(m[:], ml[:], gum[:, 4 * t:4 * t + 4], op=A.add)
            nmx = W.tile([B, 1], f32, tag="nmx")
            nc.vector.tensor_reduce(nmx[:], m[:], axis=mybir.AxisListType.X,
                                    op=A.max, negate=True)
            oh = W.tile([B, 4], f32, tag="oh")
            nc.vector.tensor_scalar(oh[:], m[:], nmx[:], 0.0, op0=A.add,
                                    op1=A.is_ge)
            # ---- logp ----
            nmlx = W.tile([B, 1], f32, tag="nmlx")
            nc.vector.tensor_reduce(nmlx[:], ml[:], axis=mybir.AxisListType.X,
                                    op=A.max, negate=True)
            e4 = W.tile([B, 4], f32, tag="e4")
            sse = W.tile([B, 1], f32, tag="sse")
            nc.scalar.activation(e4[:], ml[:], F.Exp, bias=nmlx[:],
                                 accum_out=sse[:])
            lse = W.tile([B, 1], f32, tag="lse")
            nc.scalar.activation(lse[:], sse[:], F.Ln)
            mls = W.tile([B, 1], f32, tag="mls")
            ohml = W.tile([B, 4], f32, tag="ohml")
            nc.vector.scalar_tensor_tensor(ohml[:], oh[:], 1.0, ml[:],
                                           op0=A.mult, op1=A.mult,
                                           accum_out=mls[:])
            lgp2 = W.tile([B, 1], f32, tag="lgp2")
            nc.vector.scalar_tensor_tensor(lgp2[:], mls[:], nmlx[:], lgp[:],
                                           op0=A.add, op1=A.add)
            nc.vector.tensor_scalar(lgp[:], lgp2[:], lse[:], None, op0=A.subtract)
            # ---- token bits / state update ----
            nc.vector.tensor_reduce(cfg[:, t:t + 1], oh[:, 2:4],
                                    axis=mybir.AxisListType.X, op=A.add)
            ohodd = oh[:].rearrange("p (a b) -> p a b", a=2)[:, :, 1]
            nc.vector.tensor_reduce(cfg[:, N_ORB + t:N_ORB + t + 1], ohodd,
                                    axis=mybir.AxisListType.X, op=A.add)
            nc.vector.tensor_scalar(a_rem[:], a_rem[:], cfg[:, t:t + 1], None,
                                    op0=A.subtract)
            nc.vector.tensor_scalar(b_rem[:], b_rem[:],
                                    cfg[:, N_ORB + t:N_ORB + t + 1], None,
                                    op0=A.subtract)
            # ---- next token embedding ----
            if t < N_ORB - 1:
                ohT_ps = transpose_pe(oh[:], B, 4)
                ohT = W.tile([4, B], f32, tag="ohT")
                nc.scalar.copy(ohT[:], ohT_ps[:4, :B])
                xe_ps = P.tile([D, B], f32, tag="ps")
                nc.tensor.matmul(xe_ps[:], emb4[:], ohT[:])
                xeT = W.tile([D, B], f32, tag="xeT")
                nc.vector.tensor_scalar_add(xeT[:], xe_ps[:],
                                            posT[:, t + 1:t + 2])
                xb_ps = transpose_pe(xeT[:], D, B)
                xnext = S.tile([B, D], f32, tag="x")
                nc.vector.tensor_copy(xnext[:], xb_ps[:B, :D])
                x = xnext

        nc.sync.dma_start(cfg_out.ap(), cfg[:])
        nc.sync.dma_start(lgp_out.ap(), lgp[:])

    nc.compile()
    return nc


def _prepare_inputs_per_core(inputs):
    """Host-side folds + per-core input maps."""
    f64 = np.float64
    state_emb = np.asarray(inputs["state_emb"], f64)
    pos_emb = np.asarray(inputs["pos_emb"], f64)
    ln1_w = np.asarray(inputs["ln1_w"], f64); ln1_b = np.asarray(inputs["ln1_b"], f64)
    in_w = np.asarray(inputs["in_proj_w"], f64); in_b = np.asarray(inputs["in_proj_b"], f64)
    out_w = np.asarray(inputs["out_proj_w"], f64); out_b = np.asarray(inputs["out_proj_b"], f64)
    ln2_w = np.asarray(inputs["ln2_w"], f64); ln2_b = np.asarray(inputs["ln2_b"], f64)
    ffn_w1 = np.asarray(inputs["ffn_w1"], f64); ffn_b1 = np.asarray(inputs["ffn_b1"], f64)
    ffn_w2 = np.asarray(inputs["ffn_w2"], f64); ffn_b2 = np.asarray(inputs["ffn_b2"], f64)
    fn_w = np.asarray(inputs["fn_w"], f64); fn_b = np.asarray(inputs["fn_b"], f64)
    head_w = np.asarray(inputs["head_w"], f64); head_b = np.asarray(inputs["head_b"], f64)

    com = {}
    com["wqkv"] = np.stack([(in_w[l] * ln1_w[l][None, :]).T for l in range(L)])
    com["bqkv"] = np.stack([(in_b[l] + in_w[l] @ ln1_b[l]).reshape(3, D).T
                            for l in range(L)])
    com["wout"] = np.stack([out_w[l].T for l in range(L)])
    com["bout"] = np.stack([out_b[l][:, None] for l in range(L)])
    com["wff1"] = np.stack([(ffn_w1[l] * ln2_w[l][None, :]).T for l in range(L)])
    com["bff1"] = np.stack([(ffn_b1[l] + ffn_w1[l] @ ln2_b[l]).reshape(4, D).T
                            for l in range(L)])
    com["wff2"] = np.stack([ffn_w2[l].T.reshape(4, D, D) for l in range(L)])
    com["bff2"] = np.stack([ffn_b2[l][:, None] for l in range(L)])
    com["whead"] = (head_w * fn_w[None, :]).T
    com["bheadc"] = (head_b + head_w @ fn_b)[:, None]
    com["emb4"] = state_emb[:4]
    com["posT"] = pos_emb[:N_ORB].T  # col t = pos_emb[t]; col 0 unused
    com["cA"] = np.broadcast_to(np.array([0, 0, 1, 1], f64), (B, 4)).copy()
    com["cB"] = np.broadcast_to(np.array([0, 1, 0, 1], f64), (B, 4)).copy()
    com["iota4"] = np.broadcast_to(np.arange(4, dtype=f64), (B, 4)).copy()
    com["ident"] = np.eye(D)
    x0 = state_emb[BOS] + pos_emb[0]
    com = {k: np.ascontiguousarray(v, np.float32) for k, v in com.items()}

    # gumbel noise exactly as the reference draws it (reference only runs on
    # the CPU backend in this environment, so match CPU RNG lowering)
    import jax
    with jax.default_device(jax.devices("cpu")[0]):
        keys = jax.random.split(jax.random.key(42), N_ORB)
        g = np.stack([np.asarray(jax.random.gumbel(k, (NCORES * B, 4),
                                                   dtype=np.float32)) for k in keys])

    maps = []
    for c in range(NCORES):
        m = dict(com)
        m["x0r"] = np.ascontiguousarray(
            np.broadcast_to(x0.astype(np.float32), (B, D)))
        gs = g[:, c * B:(c + 1) * B, :]          # [32, 128, 4]
        m["gum"] = np.ascontiguousarray(gs.transpose(1, 0, 2).reshape(B, 4 * N_ORB))
        maps.append(m)
    return maps


def kernel(**inputs):
    n = int(np.asarray(inputs["n_samples"]))
    assert n == NCORES * B, f"kernel compiled for n_samples=1024, got {n}"
    if "prog" not in _PROG_CACHE:
        _PROG_CACHE["prog"] = _build_program()
    nc = _PROG_CACHE["prog"]
    maps = _prepare_inputs_per_core(inputs)
    from concourse.bass_utils import run_bass_kernel_spmd
    res = run_bass_kernel_spmd(nc, maps, list(range(NCORES)))
    cfgs = np.concatenate([res.results[c]["cfg"] for c in range(NCORES)], axis=0)
    lgps = np.concatenate([res.results[c]["lgp"][:, 0] for c in range(NCORES)],
                          axis=0)
    return cfgs.astype(np.float32), lgps.astype(np.float32)


# revision 17
# speedup vs baseline: 1.0071x; 1.0071x over previous
"""Trainium2 Bass kernel for an autoregressive transformer sampler.

Model: 32-step incremental decode, 4 transformer layers, d_model=128,
4 heads x 32 head-dim, ffn 512, vocab-4 head with electron-budget
validity masking and Gumbel-max categorical sampling.

Sharding: pure data parallel. 1024 samples -> 8 cores x 128 samples.
On-core layout: batch-major activations (sample on the SBUF partition
axis), feature-major weights; all matmuls on the PE via transposes,
attention elementwise streams split between GpSimd (muls) and Vector
(segmented reduces), transcendentals on the Scalar engine.
"""

import os
import numpy as np

ATT_MUL_ENGINE = os.environ.get("ATT_MUL_ENGINE", "gpsimd")

N_ORB = 32
D = 128
H = 4
HD = 32
L = 4
DFF = 512
NA, NB = 16, 16
BOS = 4
B = 128          # samples per core
NCORES = 8
NEG = -1e9
SCALE = 1.0 / np.sqrt(HD)

_PROG_CACHE = {}


def _build_program():
    from contextlib import ExitStack
    import concourse.bass as bass
    import concourse.tile as tile
    from concourse import bacc, mybir

    F = mybir.ActivationFunctionType
    A = mybir.AluOpType
    f32 = mybir.dt.float32

    nc = bacc.Bacc("TRN2", target_bir_lowering=False, debug=False)
    mul_eng = nc.gpsimd if ATT_MUL_ENGINE == "gpsimd" else nc.vector

    din = {}
    def dram_in(name, shape):
        din[name] = nc.dram_tensor(name, list(shape), f32, kind="ExternalInput")
        return din[name]

    dram_in("wqkv", (L, D, 3 * D))
    dram_in("bqkv", (L, D, 3))
    dram_in("wout", (L, D, D))
    dram_in("bout", (L, D, 1))
    dram_in("wff1", (L, D, DFF))
    dram_in("bff1", (L, D, 4))
    dram_in("wff2", (L, 4, D, D))
    dram_in("bff2", (L, D, 1))
    dram_in("whead", (D, 4))
    dram_in("bheadc", (4, 1))
    dram_in("emb4", (4, D))
    dram_in("x0r", (B, D))
    dram_in("posT", (D, N_ORB))
    dram_in("gum", (B, 4 * N_ORB))
    dram_in("cA", (B, 4))
    dram_in("cB", (B, 4))
    dram_in("iota4", (B, 4))
    dram_in("ident", (D, D))

    cfg_out = nc.dram_tensor("cfg", [B, 2 * N_ORB], f32, kind="ExternalOutput")
    lgp_out = nc.dram_tensor("lgp", [B, 1], f32, kind="ExternalOutput")

    with tile.TileContext(nc) as tc, ExitStack() as ctx:
        S = ctx.enter_context(tc.tile_pool(name="singles", bufs=1))
        W = ctx.enter_context(tc.tile_pool(name="work", bufs=2))
        P = ctx.enter_context(tc.tile_pool(name="psum", bufs=3, space="PSUM"))
        PF = ctx.enter_context(tc.tile_pool(name="psumff", bufs=2, space="PSUM"))

        # ---- load constants / weights ----
        def load(name, shape, src_ap):
            t = S.tile(list(shape), f32, tag=name, name=name)
            nc.sync.dma_start(t[:], src_ap)
            return t

        wqkv = [load(f"wqkv{l}", (D, 3 * D), din["wqkv"].ap()[l]) for l in range(L)]
        bqkv = [load(f"bqkv{l}", (D, 3), din["bqkv"].ap()[l]) for l in range(L)]
        wout = [load(f"wout{l}", (D, D), din["wout"].ap()[l]) for l in range(L)]
        bout = [load(f"bout{l}", (D, 1), din["bout"].ap()[l]) for l in range(L)]
        wff1 = [load(f"wff1{l}", (D, DFF), din["wff1"].ap()[l]) for l in range(L)]
        bff1 = [load(f"bff1{l}", (D, 4), din["bff1"].ap()[l]) for l in range(L)]
        wff2 = [[load(f"wff2{l}_{c}", (D, D), din["wff2"].ap()[l, c]) for c in range(4)]
                for l in range(L)]
        bff2 = [load(f"bff2{l}", (D, 1), din["bff2"].ap()[l]) for l in range(L)]
        whead = load("whead", (D, 4), din["whead"].ap())
        bheadc = load("bheadc", (4, 1), din["bheadc"].ap())
        emb4 = load("emb4", (4, D), din["emb4"].ap())
        posT = load("posT", (D, N_ORB), din["posT"].ap())
        gum = load("gum", (B, 4 * N_ORB), din["gum"].ap())
        cA = load("cA", (B, 4), din["cA"].ap())
        cB = load("cB", (B, 4), din["cB"].ap())
        iota4 = load("iota4", (B, 4), din["iota4"].ap())
        ident = load("ident", (D, D), din["ident"].ap())

        # ---- persistent state ----
        Kc = [S.tile([B, H * N_ORB * HD], f32, tag=f"Kc{l}", name=f"Kc{l}") for l in range(L)]
        Vc = [S.tile([B, H * HD * N_ORB], f32, tag=f"Vc{l}", name=f"Vc{l}") for l in range(L)]
        tmp1 = S.tile([B, H * N_ORB * HD], f32, tag="tmp1")
        tmp2 = tmp1
        cfg = S.tile([B, 2 * N_ORB], f32, tag="cfg")
        lgp = S.tile([B, 1], f32, tag="lgp")
        a_rem = S.tile([B, 1], f32, tag="a_rem")
        b_rem = S.tile([B, 1], f32, tag="b_rem")
        x = S.tile([B, D], f32, tag="x")

        nc.vector.memset(lgp[:], 0.0)
        nc.vector.memset(a_rem[:], float(NA))
        nc.vector.memset(b_rem[:], float(NB))
        nc.sync.dma_start(x[:], din["x0r"].ap())

        def transpose_pe(src_sb, p, f):
            """src [p, f] sbuf -> psum [f, p]"""
            t = P.tile([128, 128], f32, tag="ps", name="tps")
            nc.tensor.matmul(t[:f, :p], src_sb, ident[:p, :p], is_transpose=True)
            return t

        def layernorm_stats(xin):
            """xin [B, D] sbuf -> rstd [B,1] (1 Newton step), mean [B,1]"""
            st6 = W.tile([B, 6], f32, tag="st6")
            nc.vector.bn_stats(st6[:], xin[:])
            mv = W.tile([B, 2], f32, tag="mv")
            nc.vector.bn_aggr(mv[:], st6[:])
            mean = mv[:, 0:1]
            vp = W.tile([B, 1], f32, tag="vp")
            nc.vector.tensor_scalar(vp[:], mv[:, 1:2], 1e-5, None, op0=A.add)
            s = W.tile([B, 1], f32, tag="lns")
            nc.scalar.activation(s[:], vp[:], F.Sqrt)
            r0 = W.tile([B, 1], f32, tag="lnr0")
            nc.vector.reciprocal(r0[:], s[:])
            r2 = W.tile([B, 1], f32, tag="lnr2")
            nc.scalar.activation(r2[:], r0[:], F.Square)
            w1 = W.tile([B, 1], f32, tag="lnw1")
            nc.vector.tensor_scalar(w1[:], r2[:], vp[:], -0.5, op0=A.mult, op1=A.mult)
            rstd = W.tile([B, 1], f32, tag="lnrstd")
            nc.vector.scalar_tensor_tensor(rstd[:], w1[:], 1.5, r0[:],
                                           op0=A.add, op1=A.mult)
            return mean, rstd

        def normalize_T(xin):
            """ln(xin) (w/b folded into weights) -> sbuf [D, B] feature-major"""
            mean, rstd = layernorm_stats(xin)
            xn = W.tile([B, D], f32, tag="xn", name="xn")
            nc.vector.tensor_scalar(xn[:], xin[:], mean, rstd, op0=A.subtract,
                                    op1=A.mult)
            xnT_ps = transpose_pe(xn[:], B, D)
            xnT = W.tile([D, B], f32, tag="xnT", name="xnT")
            nc.scalar.copy(xnT[:], xnT_ps[:D, :B])
            return xnT

        for t in range(N_ORB):
            for l in range(L):
                # ---- LN1 + qkv ----
                h1T = normalize_T(x[:])
                qkvT_ps = P.tile([D, 3 * B], f32, tag="ps")
                for c in range(3):
                    nc.tensor.matmul(qkvT_ps[:, c * B:(c + 1) * B],
                                     wqkv[l][:, c * D:(c + 1) * D], h1T[:])
                # bias per m-partition, move to sbuf feature-major
                qkvT = W.tile([D, 3 * B], f32, tag="qkvT")
                for c in range(3):
                    nc.vector.tensor_scalar_add(
                        qkvT[:, c * B:(c + 1) * B],
                        qkvT_ps[:, c * B:(c + 1) * B],
                        bqkv[l][:, c:c + 1])
                # transpose each to batch-major
                q_bm = W.tile([B, D], f32, tag="q_bm")
                qps = transpose_pe(qkvT[:, 0 * B:1 * B], D, B)
                nc.scalar.copy(q_bm[:], qps[:B, :D])
                kps = transpose_pe(qkvT[:, 1 * B:2 * B], D, B)
                Kc4 = Kc[l][:].rearrange("p (h t d) -> p h t d", h=H, t=N_ORB)
                nc.vector.tensor_copy(
                    Kc4[:, :, t, :], kps[:B, :D].rearrange("p (h d) -> p h d", h=H))
                vps = transpose_pe(qkvT[:, 2 * B:3 * B], D, B)
                Vc4 = Vc[l][:].rearrange("p (h d t) -> p h d t", h=H, d=HD)
                nc.vector.tensor_copy(
                    Vc4[:, :, :, t], vps[:B, :D].rearrange("p (h d) -> p h d", h=H))

                # ---- attention (per-head: stt/reduce are limited to 3D) ----
                T1 = t + 1
                tmp1v = tmp1[:].rearrange("p (h t d) -> p h t d", h=H, t=N_ORB)
                tmp2v = tmp2[:].rearrange("p (h d t) -> p h d t", h=H, d=HD)
                qh = q_bm[:].rearrange("p (h d) -> p h d", h=H)
                sc = W.tile([B, H * N_ORB], f32, tag="sc")
                sc3 = sc[:].rearrange("p (h t) -> p h t", h=H)[:, :, :T1]
                ee = W.tile([B, H * N_ORB], f32, tag="ee")
                ee3 = ee[:].rearrange("p (h t) -> p h t", h=H)[:, :, :T1]
                for h in range(H):
                    t1v = tmp1v[:, h, :T1, :]
                    qb = qh[:, h, :].unsqueeze(1).broadcast_to([B, T1, HD])
                    mul_eng.tensor_tensor(t1v, Kc4[:, h, :T1, :], qb,
                                          op=A.mult)
                    nc.vector.tensor_reduce(sc3[:, h, :], t1v,
                                            axis=mybir.AxisListType.X, op=A.add)
                nc.scalar.activation(ee3, sc3, F.Exp, scale=float(SCALE))
                se = W.tile([B, H], f32, tag="se")
                nc.vector.tensor_reduce(se[:].unsqueeze(2), ee3,
                                        axis=mybir.AxisListType.X, op=A.add)
                rse = W.tile([B, H], f32, tag="rse")
                nc.vector.reciprocal(rse[:], se[:])
                att = W.tile([B, D], f32, tag="att")
                att3 = att[:].rearrange("p (h d) -> p h d", h=H)
                for h in range(H):
                    t2v = tmp2v[:, h, :, :T1]
                    eb = ee3[:, h, :].unsqueeze(1).broadcast_to([B, HD, T1])
                    mul_eng.tensor_tensor(t2v, Vc4[:, h, :, :T1], eb,
                                          op=A.mult)
                    nc.vector.tensor_reduce(att3[:, h, :], t2v,
                                            axis=mybir.AxisListType.X, op=A.add)
                attn = W.tile([B, D], f32, tag="attn")
                rseb = rse[:].unsqueeze(2).broadcast_to([B, H, HD])
                nc.vector.tensor_tensor(attn[:].rearrange("p (h d) -> p h d", h=H),
                                        att3, rseb, op=A.mult)
                # ---- out proj + residual ----
                attT_ps = transpose_pe(attn[:], B, D)
                attT = W.tile([D, B], f32, tag="attT")
                nc.scalar.copy(attT[:], attT_ps[:D, :B])
                oT_ps = P.tile([D, B], f32, tag="ps")
                nc.tensor.matmul(oT_ps[:], wout[l][:], attT[:])
                oT = W.tile([D, B], f32, tag="oT")
                nc.vector.tensor_scalar_add(oT[:], oT_ps[:], bout[l][:])
                o_ps = transpose_pe(oT[:], D, B)
                x2 = W.tile([B, D], f32, tag="x2")
                nc.vector.tensor_tensor(x2[:], x[:], o_ps[:B, :D], op=A.add)

                # ---- LN2 + FFN ----
                h2T = normalize_T(x2[:])
                g1c = []
                for c in range(4):
                    f1_ps = PF.tile([D, B], f32, tag="f1ps")
                    nc.tensor.matmul(f1_ps[:], wff1[l][:, c * D:(c + 1) * D], h2T[:])
                    g1 = W.tile([D, B], f32, tag=f"g1_{c}", name=f"g1_{c}")
                    nc.scalar.activation(g1[:], f1_ps[:],
                                         F.Gelu, bias=bff1[l][:, c:c + 1])
                    g1c.append(g1)
                f2_ps = P.tile([D, B], f32, tag="f2ps", bufs=1, name="f2_ps")
                for c in range(4):
                    nc.tensor.matmul(f2_ps[:], wff2[l][c][:], g1c[c][:],
                                     start=(c == 0), stop=(c == 3))
                f2T = W.tile([D, B], f32, tag="f2T")
                nc.vector.tensor_scalar_add(f2T[:], f2_ps[:], bff2[l][:])
                f2b_ps = transpose_pe(f2T[:], D, B)
                xnew = S.tile([B, D], f32, tag="x")
                nc.vector.tensor_tensor(xnew[:], x2[:], f2b_ps[:B, :D], op=A.add)
                x = xnew

            # ---- head ----
            xfT = normalize_T(x[:])
            lg_ps = P.tile([4, B], f32, tag="ps")
            nc.tensor.matmul(lg_ps[:], whead[:], xfT[:])
            lgT = W.tile([4, B], f32, tag="lgT")
            nc.scalar.activation(lgT[:], lg_ps[:], F.Identity, bias=bheadc[:])
            lg_ps2 = transpose_pe(lgT[:], 4, B)
            # ---- validity mask ----
            after = float(N_ORB - 1 - t)
            arm = W.tile([B, 1], f32, tag="arm")
            nc.vector.tensor_scalar_add(arm[:], a_rem[:], -after)
            brm = W.tile([B, 1], f32, tag="brm")
            nc.vector.tensor_scalar_add(brm[:], b_rem[:], -after)
            va = W.tile([B, 4], f32, tag="va")
            vb = W.tile([B, 4], f32, tag="vb")
            va2 = W.tile([B, 4], f32, tag="va2")
            nc.vector.tensor_scalar(va[:], cA[:], a_rem[:], None, op0=A.is_le)
            nc.vector.tensor_scalar(va2[:], cA[:], arm[:], None, op0=A.is_ge)
            nc.vector.tensor_tensor(va[:], va[:], va2[:], op=A.mult)
            vb2 = W.tile([B, 4], f32, tag="vb2")
            nc.vector.tensor_scalar(vb[:], cB[:], b_rem[:], None, op0=A.is_le)
            nc.vector.tensor_scalar(vb2[:], cB[:], brm[:], None, op0=A.is_ge)
            nc.vector.tensor_tensor(vb[:], vb[:], vb2[:], op=A.mult)
            valid = W.tile([B, 4], f32, tag="valid")
            nc.vector.tensor_tensor(valid[:], va[:], vb[:], op=A.mult)
            vm1 = W.tile([B, 4], f32, tag="vm1")
            nc.vector.tensor_scalar(vm1[:], valid[:], 1.0, None, op0=A.subtract)
            ml = W.tile([B, 4], f32, tag="ml")
            nc.vector.scalar_tensor_tensor(ml[:], vm1[:], 1e9, lg_ps2[:B, :4],
                                           op0=A.mult, op1=A.add)
            # ---- gumbel argmax ----
            m = W.tile([B, 4], f32, tag="m")
            nc.vector.tensor_tensor(m[:], ml[:], gum[:, 4 * t:4 * t + 4], op=A.add)
            nmx = W.tile([B, 1], f32, tag="nmx")
            nc.vector.tensor_reduce(nmx[:], m[:], axis=mybir.AxisListType.X,
                                    op=A.max, negate=True)
            oh = W.tile([B, 4], f32, tag="oh")
            nc.vector.tensor_scalar(oh[:], m[:], nmx[:], 0.0, op0=A.add,
                                    op1=A.is_ge)
            # ---- logp ----
            nmlx = W.tile([B, 1], f32, tag="nmlx")
            nc.vector.tensor_reduce(nmlx[:], ml[:], axis=mybir.AxisListType.X,
                                    op=A.max, negate=True)
            e4 = W.tile([B, 4], f32, tag="e4")
            sse = W.tile([B, 1], f32, tag="sse")
            nc.scalar.activation(e4[:], ml[:], F.Exp, bias=nmlx[:],
                                 accum_out=sse[:])
            lse = W.tile([B, 1], f32, tag="lse")
            nc.scalar.activation(lse[:], sse[:], F.Ln)
            mls = W.tile([B, 1], f32, tag="mls")
            ohml = W.tile([B, 4], f32, tag="ohml")
            nc.vector.scalar_tensor_tensor(ohml[:], oh[:], 1.0, ml[:],
                                           op0=A.mult, op1=A.mult,
                                           accum_out=mls[:])
            lgp2 = W.tile([B, 1], f32, tag="lgp2")
            nc.vector.scalar_tensor_tensor(lgp2[:], mls[:], nmlx[:], lgp[:],
                                           op0=A.add, op1=A.add)
            nc.vector.tensor_scalar(lgp[:], lgp2[:], lse[:], None, op0=A.subtract)
            # ---- token bits / state update ----
            nc.vector.tensor_reduce(cfg[:, t:t + 1], oh[:, 2:4],
                                    axis=mybir.AxisListType.X, op=A.add)
            ohodd = oh[:].rearrange("p (a b) -> p a b", a=2)[:, :, 1]
            nc.vector.tensor_reduce(cfg[:, N_ORB + t:N_ORB + t + 1], ohodd,
                                    axis=mybir.AxisListType.X, op=A.add)
            nc.vector.tensor_scalar(a_rem[:], a_rem[:], cfg[:, t:t + 1], None,
                                    op0=A.subtract)
            nc.vector.tensor_scalar(b_rem[:], b_rem[:],
                                    cfg[:, N_ORB + t:N_ORB + t + 1], None,
                                    op0=A.subtract)
            # ---- next token embedding ----
            if t < N_ORB - 1:
                ohT_ps = transpose_pe(oh[:], B, 4)
                ohT = W.tile([4, B], f32, tag="ohT")
                nc.scalar.copy(ohT[:], ohT_ps[:4, :B])
                xe_ps = P.tile([D, B], f32, tag="ps")
                nc.tensor.matmul(xe_ps[:], emb4[:], ohT[:])
                xeT = W.tile([D, B], f32, tag="xeT")
                nc.vector.tensor_scalar_add(xeT[:], xe_ps[:],
                                            posT[:, t + 1:t + 2])
                xb_ps = transpose_pe(xeT[:], D, B)
                xnext = S.tile([B, D], f32, tag="x")
                nc.vector.tensor_copy(xnext[:], xb_ps[:B, :D])
                x = xnext

        nc.sync.dma_start(cfg_out.ap(), cfg[:])
        nc.sync.dma_start(lgp_out.ap(), lgp[:])

    nc.compile()
    return nc


def _prepare_inputs_per_core(inputs):
    """Host-side folds + per-core input maps."""
    f64 = np.float64
    state_emb = np.asarray(inputs["state_emb"], f64)
    pos_emb = np.asarray(inputs["pos_emb"], f64)
    ln1_w = np.asarray(inputs["ln1_w"], f64); ln1_b = np.asarray(inputs["ln1_b"], f64)
    in_w = np.asarray(inputs["in_proj_w"], f64); in_b = np.asarray(inputs["in_proj_b"], f64)
    out_w = np.asarray(inputs["out_proj_w"], f64); out_b = np.asarray(inputs["out_proj_b"], f64)
    ln2_w = np.asarray(inputs["ln2_w"], f64); ln2_b = np.asarray(inputs["ln2_b"], f64)
    ffn_w1 = np.asarray(inputs["ffn_w1"], f64); ffn_b1 = np.asarray(inputs["ffn_b1"], f64)
    ffn_w2 = np.asarray(inputs["ffn_w2"], f64); ffn_b2 = np.asarray(inputs["ffn_b2"], f64)
    fn_w = np.asarray(inputs["fn_w"], f64); fn_b = np.asarray(inputs["fn_b"], f64)
    head_w = np.asarray(inputs["head_w"], f64); head_b = np.asarray(inputs["head_b"], f64)

    com = {}
    com["wqkv"] = np.stack([(in_w[l] * ln1_w[l][None, :]).T for l in range(L)])
    com["bqkv"] = np.stack([(in_b[l] + in_w[l] @ ln1_b[l]).reshape(3, D).T
                            for l in range(L)])
    com["wout"] = np.stack([out_w[l].T for l in range(L)])
    com["bout"] = np.stack([out_b[l][:, None] for l in range(L)])
    com["wff1"] = np.stack([(ffn_w1[l] * ln2_w[l][None, :]).T for l in range(L)])
    com["bff1"] = np.stack([(ffn_b1[l] + ffn_w1[l] @ ln2_b[l]).reshape(4, D).T
                            for l in range(L)])
    com["wff2"] = np.stack([ffn_w2[l].T.reshape(4, D, D) for l in range(L)])
    com["bff2"] = np.stack([ffn_b2[l][:, None] for l in range(L)])
    com["whead"] = (head_w * fn_w[None, :]).T
    com["bheadc"] = (head_b + head_w @ fn_b)[:, None]
    com["emb4"] = state_emb[:4]
    com["posT"] = pos_emb[:N_ORB].T  # col t = pos_emb[t]; col 0 unused
    com["cA"] = np.broadcast_to(np.array([0, 0, 1, 1], f64), (B, 4)).copy()
    com["cB"] = np.broadcast_to(np.array([0, 1, 0, 1], f64), (B, 4)).copy()
    com["iota4"] = np.broadcast_to(np.arange(4, dtype=f64), (B, 4)).copy()
    com["ident"] = np.eye(D)
    x0 = state_emb[BOS] + pos_emb[0]
    com = {k: np.ascontiguousarray(v, np.float32) for k, v in com.items()}

    # gumbel noise exactly as the reference draws it (reference only runs on
    # the CPU backend in this environment, so match CPU RNG lowering)
    import jax
    with jax.default_device(jax.devices("cpu")[0]):
        keys = jax.random.split(jax.random.key(42), N_ORB)
        g = np.stack([np.asarray(jax.random.gumbel(k, (NCORES * B, 4),
                                                   dtype=np.float32)) for k in keys])

    maps = []
    for c in range(NCORES):
        m = dict(com)
        m["x0r"] = np.ascontiguousarray(
            np.broadcast_to(x0.astype(np.float32), (B, D)))
        gs = g[:, c * B:(c + 1) * B, :]          # [32, 128, 4]
        m["gum"] = np.ascontiguousarray(gs.transpose(1, 0, 2).reshape(B, 4 * N_ORB))
        maps.append(m)
    return maps


def kernel(**inputs):
    n = int(np.asarray(inputs["n_samples"]))
    assert n == NCORES * B, f"kernel compiled for n_samples=1024, got {n}"
    if "prog" not in _PROG_CACHE:
        _PROG_CACHE["prog"] = _build_program()
    nc = _PROG_CACHE["prog"]
    maps = _prepare_inputs_per_core(inputs)
    from concourse.bass_utils import run_bass_kernel_spmd
    res = run_bass_kernel_spmd(nc, maps, list(range(NCORES)))
    cfgs = np.concatenate([res.results[c]["cfg"] for c in range(NCORES)], axis=0)
    lgps = np.concatenate([res.results[c]["lgp"][:, 0] for c in range(NCORES)],
                          axis=0)
    return cfgs.astype(np.float32), lgps.astype(np.float32)


# revision 21
# speedup vs baseline: 1.3277x; 1.3184x over previous
"""Trainium2 Bass kernel for an autoregressive transformer sampler.

Model: 32-step incremental decode, 4 transformer layers, d_model=128,
4 heads x 32 head-dim, ffn 512, vocab-4 head with electron-budget
validity masking and Gumbel-max categorical sampling.

Sharding: pure data parallel. 1024 samples -> 8 cores x 128 samples.
On-core layout: batch-major activations (sample on the SBUF partition
axis). Matmuls run on the PE with the transposed normalized activations
as the stationary operand so outputs land batch-major directly. The KV
caches are stored time-major [sample, (t, head, dim)] so each attention
stage (QK product, segmented reduces, softmax, PV product) is a single
strided-AP instruction. Biases / position embeddings are folded into
host-precomputed replicated constants.
"""

import os
import numpy as np

N_ORB = 32
D = 128
H = 4
HD = 32
L = 4
DFF = 512
NA, NB = 16, 16
BOS = 4
B = 128          # samples per core
NCORES = 8
NEG = -1e9
SCALE = 1.0 / np.sqrt(HD)

N_STEPS = int(os.environ.get("KERNEL_STEPS", str(N_ORB)))
USE_NEWTON = os.environ.get("KERNEL_NEWTON", "1") == "1"

_PROG_CACHE = {}


def _build_program():
    from contextlib import ExitStack
    import concourse.bass as bass
    import concourse.tile as tile
    from concourse import bacc, mybir

    F = mybir.ActivationFunctionType
    A = mybir.AluOpType
    f32 = mybir.dt.float32
    X = mybir.AxisListType.X

    nc = bacc.Bacc("TRN2", target_bir_lowering=False, debug=False)

    din = {}
    def dram_in(name, shape):
        din[name] = nc.dram_tensor(name, list(shape), f32, kind="ExternalInput")
        return din[name]

    dram_in("wqkv", (L, D, 3 * D))     # lhs-side weights (feature major)
    dram_in("bqkvr", (L, B, 3 * D))    # replicated qkv bias rows
    dram_in("wout", (L, D, D))
    dram_in("boutr", (L, B, D))
    dram_in("wff1", (L, D, DFF))
    dram_in("bff1", (L, D, 4))         # per-partition bias columns per chunk
    dram_in("wff2", (L, 4, D, D))
    dram_in("bff2r", (L, B, D))
    dram_in("whead", (D, 4))
    dram_in("bhmr", (B, 4))            # folded head bias, replicated
    dram_in("embp", (4, (N_ORB - 1) * D))  # (state_emb + pos[t+1]) per step
    dram_in("x0r", (B, D))
    dram_in("gum", (B, 4 * N_ORB))     # gumbel + folded head bias
    dram_in("cA", (B, 4))
    dram_in("cB", (B, 4))
    dram_in("ident", (D, D))

    cfg_out = nc.dram_tensor("cfg", [B, 2 * N_ORB], f32, kind="ExternalOutput")
    lgp_out = nc.dram_tensor("lgp", [B, 1], f32, kind="ExternalOutput")

    with tile.TileContext(nc) as tc, ExitStack() as ctx:
        S = ctx.enter_context(tc.tile_pool(name="singles", bufs=1))
        W = ctx.enter_context(tc.tile_pool(name="work", bufs=1))
        P = ctx.enter_context(tc.tile_pool(name="psum", bufs=3, space="PSUM"))
        PF = ctx.enter_context(tc.tile_pool(name="psumff", bufs=2, space="PSUM"))

        def load(name, shape, src_ap):
            t = S.tile(list(shape), f32, tag=name, name=name)
            nc.sync.dma_start(t[:], src_ap)
            return t

        wqkv = [load(f"wqkv{l}", (D, 3 * D), din["wqkv"].ap()[l]) for l in range(L)]
        bqkvr = [load(f"bqkvr{l}", (B, 3 * D), din["bqkvr"].ap()[l]) for l in range(L)]
        wout = [load(f"wout{l}", (D, D), din["wout"].ap()[l]) for l in range(L)]
        boutr = [load(f"boutr{l}", (B, D), din["boutr"].ap()[l]) for l in range(L)]
        wff1 = [load(f"wff1{l}", (D, DFF), din["wff1"].ap()[l]) for l in range(L)]
        bff1 = [load(f"bff1{l}", (D, 4), din["bff1"].ap()[l]) for l in range(L)]
        wff2 = [[load(f"wff2{l}_{c}", (D, D), din["wff2"].ap()[l, c]) for c in range(4)]
                for l in range(L)]
        bff2r = [load(f"bff2r{l}", (B, D), din["bff2r"].ap()[l]) for l in range(L)]
        whead = load("whead", (D, 4), din["whead"].ap())
        bhmr = load("bhmr", (B, 4), din["bhmr"].ap())
        gum = load("gum", (B, 4 * N_ORB), din["gum"].ap())
        cA = load("cA", (B, 4), din["cA"].ap())
        cB = load("cB", (B, 4), din["cB"].ap())
        ident = load("ident", (D, D), din["ident"].ap())

        # persistent state
        Kc = [S.tile([B, N_ORB * D], f32, tag=f"Kc{l}", name=f"Kc{l}") for l in range(L)]
        Vc = [S.tile([B, N_ORB * D], f32, tag=f"Vc{l}", name=f"Vc{l}") for l in range(L)]
        tmp = S.tile([B, N_ORB * D], f32, tag="tmp", name="tmp")
        cfg = S.tile([B, 2 * N_ORB], f32, tag="cfg", name="cfg")
        lgp = S.tile([B, 1], f32, tag="lgp", name="lgp")
        a_rem = S.tile([B, 1], f32, tag="a_rem", name="a_rem")
        b_rem = S.tile([B, 1], f32, tag="b_rem", name="b_rem")
        x = S.tile([B, D], f32, tag="x", name="x")

        nc.vector.memset(lgp[:], 0.0)
        nc.vector.memset(a_rem[:], float(NA))
        nc.vector.memset(b_rem[:], float(NB))
        for l in range(L):
            nc.vector.memset(Kc[l][:], 0.0)
            nc.vector.memset(Vc[l][:], 0.0)
        nc.sync.dma_start(x[:], din["x0r"].ap())

        def transpose_pe(src_sb, p, f):
            t = P.tile([128, 128], f32, tag="ps", name="tps")
            nc.tensor.matmul(t[:f, :p], src_sb, ident[:p, :p], is_transpose=True)
            return t

        def normalize_T(xin):
            """layernorm(xin) -> [D, B] sbuf (gains/biases folded into weights)"""
            st6 = W.tile([B, 6], f32, tag="st6", name="st6")
            nc.vector.bn_stats(st6[:], xin[:])
            mv = W.tile([B, 2], f32, tag="mv", name="mv")
            nc.vector.bn_aggr(mv[:], st6[:])
            mean = mv[:, 0:1]
            vp = W.tile([B, 1], f32, tag="vp", name="vp")
            nc.vector.tensor_scalar(vp[:], mv[:, 1:2], 1e-5, None, op0=A.add)
            s = W.tile([B, 1], f32, tag="lns", name="lns")
            nc.scalar.activation(s[:], vp[:], F.Sqrt)
            r0 = W.tile([B, 1], f32, tag="lnr0", name="lnr0")
            nc.vector.reciprocal(r0[:], s[:])
            if USE_NEWTON:
                r2 = W.tile([B, 1], f32, tag="lnr2", name="lnr2")
                nc.vector.tensor_tensor(r2[:], r0[:], r0[:], op=A.mult)
                w1 = W.tile([B, 1], f32, tag="lnw1", name="lnw1")
                nc.vector.tensor_scalar(w1[:], r2[:], vp[:], -0.5, op0=A.mult,
                                        op1=A.mult)
                rstd = W.tile([B, 1], f32, tag="lnrstd", name="lnrstd")
                nc.vector.scalar_tensor_tensor(rstd[:], w1[:], 1.5, r0[:],
                                               op0=A.add, op1=A.mult)
            else:
                rstd = r0
            xn = W.tile([B, D], f32, tag="xn", name="xn")
            nc.vector.tensor_scalar(xn[:], xin[:], mean, rstd[:], op0=A.subtract,
                                    op1=A.mult)
            xnT_ps = transpose_pe(xn[:], B, D)
            xnT = W.tile([D, B], f32, tag="xnT", name="xnT")
            nc.vector.tensor_copy(xnT[:], xnT_ps[:D, :B])
            return xnT

        for t in range(N_STEPS):
            T1 = t + 1
            for l in range(L):
                # ---- LN1 + qkv (batch-major out) ----
                h1T = normalize_T(x[:])
                qkv_ps = P.tile([B, 3 * D], f32, tag="ps", name="qkv_ps")
                nc.tensor.matmul(qkv_ps[:], h1T[:], wqkv[l][:])
                qkvb = W.tile([B, 3 * D], f32, tag="qkvb", name="qkvb")
                nc.vector.tensor_tensor(qkvb[:], qkv_ps[:], bqkvr[l][:], op=A.add)
                q = qkvb[:, 0:D]
                nc.vector.tensor_copy(Kc[l][:, t * D:(t + 1) * D], qkvb[:, D:2 * D])
                nc.vector.tensor_copy(Vc[l][:, t * D:(t + 1) * D], qkvb[:, 2 * D:3 * D])

                # ---- attention (time-major cache, whole-cache single ops) ----
                t3 = tmp[:].rearrange("p (t hd) -> p t hd", t=N_ORB)
                nc.vector.tensor_tensor(
                    t3, Kc[l][:].rearrange("p (t hd) -> p t hd", t=N_ORB),
                    q.unsqueeze(1).broadcast_to([B, N_ORB, D]), op=A.mult)
                sc = W.tile([B, N_ORB * H], f32, tag="sc", name="sc")
                nc.vector.tensor_reduce(
                    sc[:],
                    tmp[:].rearrange("p (th d) -> p th d", d=HD), axis=X, op=A.add)
                ee = W.tile([B, N_ORB * H], f32, tag="ee", name="ee")
                nc.scalar.activation(
                    ee[:].rearrange("p (t h) -> p t h", h=H)[:, :T1, :],
                    sc[:].rearrange("p (t h) -> p t h", h=H)[:, :T1, :],
                    F.Exp, scale=float(SCALE))
                se = W.tile([B, H], f32, tag="se", name="se")
                nc.vector.tensor_reduce(
                    se[:].unsqueeze(2),
                    ee[:].rearrange("p (t h) -> p h t", h=H)[:, :, :T1],
                    axis=X, op=A.add)
                rse = W.tile([B, H], f32, tag="rse", name="rse")
                nc.vector.reciprocal(rse[:], se[:])
                nc.vector.tensor_tensor(
                    t3, Vc[l][:].rearrange("p (t hd) -> p t hd", t=N_ORB),
                    ee[:].unsqueeze(2).broadcast_to([B, N_ORB * H, HD]), op=A.mult)
                att = W.tile([B, D], f32, tag="att", name="att")
                nc.vector.tensor_reduce(
                    att[:],
                    tmp[:].rearrange("p (t hd) -> p hd t", hd=D)[:, :, :T1],
                    axis=X, op=A.add)
                attn = W.tile([B, D], f32, tag="attn", name="attn")
                nc.vector.tensor_tensor(
                    attn[:].rearrange("p (h d) -> p h d", h=H),
                    att[:].rearrange("p (h d) -> p h d", h=H),
                    rse[:].unsqueeze(2).broadcast_to([B, H, HD]), op=A.mult)

                # ---- out proj + residual ----
                attT_ps = transpose_pe(attn[:], B, D)
                attT = W.tile([D, B], f32, tag="attT", name="attT")
                nc.vector.tensor_copy(attT[:], attT_ps[:D, :B])
                o_ps = P.tile([B, D], f32, tag="ps", name="o_ps")
                nc.tensor.matmul(o_ps[:], attT[:], wout[l][:])
                ob = W.tile([B, D], f32, tag="ob", name="ob")
                nc.vector.tensor_tensor(ob[:], o_ps[:], boutr[l][:], op=A.add)
                x2 = W.tile([B, D], f32, tag="x2", name="x2")
                nc.vector.tensor_tensor(x2[:], x[:], ob[:], op=A.add)

                # ---- LN2 + FFN ----
                h2T = normalize_T(x2[:])
                g1c = []
                for c in range(4):
                    f1_ps = PF.tile([D, B], f32, tag="f1ps", name="f1_ps")
                    nc.tensor.matmul(f1_ps[:], wff1[l][:, c * D:(c + 1) * D], h2T[:])
                    g1 = W.tile([D, B], f32, tag=f"g1_{c}", name=f"g1_{c}")
                    nc.scalar.activation(g1[:], f1_ps[:], F.Gelu,
                                         bias=bff1[l][:, c:c + 1])
                    g1c.append(g1)
                f2_ps = P.tile([B, D], f32, tag="f2ps", bufs=1, name="f2_ps")
                for c in range(4):
                    nc.tensor.matmul(f2_ps[:], g1c[c][:], wff2[l][c][:],
                                     start=(c == 0), stop=(c == 3))
                f2b = W.tile([B, D], f32, tag="f2b", name="f2b")
                nc.vector.tensor_tensor(f2b[:], f2_ps[:], bff2r[l][:], op=A.add)
                xnew = S.tile([B, D], f32, tag="x", name="x")
                nc.vector.tensor_tensor(xnew[:], x2[:], f2b[:], op=A.add)
                x = xnew

            # ---- head + sampling ----
            xfT = normalize_T(x[:])
            lg_ps = P.tile([B, 4], f32, tag="ps", name="lg_ps")
            nc.tensor.matmul(lg_ps[:], xfT[:], whead[:])
            after = float(N_ORB - 1 - t)
            c2a = W.tile([B, 4], f32, tag="c2a", name="c2a")
            nc.vector.tensor_scalar(c2a[:], cA[:], after, a_rem[:], op0=A.add,
                                    op1=A.is_ge)
            va = W.tile([B, 4], f32, tag="va", name="va")
            nc.vector.scalar_tensor_tensor(va[:], cA[:], a_rem[:], c2a[:],
                                           op0=A.is_le, op1=A.mult)
            c2b = W.tile([B, 4], f32, tag="c2b", name="c2b")
            nc.vector.tensor_scalar(c2b[:], cB[:], after, b_rem[:], op0=A.add,
                                    op1=A.is_ge)
            vb = W.tile([B, 4], f32, tag="vb", name="vb")
            nc.vector.scalar_tensor_tensor(vb[:], cB[:], b_rem[:], c2b[:],
                                           op0=A.is_le, op1=A.mult)
            valid = W.tile([B, 4], f32, tag="valid", name="valid")
            nc.vector.tensor_tensor(valid[:], va[:], vb[:], op=A.mult)
            vm1 = W.tile([B, 4], f32, tag="vm1", name="vm1")
            nc.vector.tensor_scalar(vm1[:], valid[:], 1.0, 1e9, op0=A.subtract,
                                    op1=A.mult)
            u = W.tile([B, 4], f32, tag="u", name="u")
            nc.vector.tensor_tensor(u[:], vm1[:], lg_ps[:], op=A.add)
            ml = W.tile([B, 4], f32, tag="ml", name="ml")
            nc.vector.tensor_tensor(ml[:], u[:], bhmr[:], op=A.add)
            m = W.tile([B, 4], f32, tag="m", name="m")
            nc.vector.tensor_tensor(m[:], u[:], gum[:, 4 * t:4 * t + 4], op=A.add)
            nmx = W.tile([B, 1], f32, tag="nmx", name="nmx")
            nc.vector.tensor_reduce(nmx[:], m[:], axis=X, op=A.max, negate=True)
            oh = W.tile([B, 4], f32, tag="oh", name="oh")
            nc.vector.tensor_scalar(oh[:], m[:], nmx[:], 0.0, op0=A.add,
                                    op1=A.is_ge)
            # logp
            nmlx = W.tile([B, 1], f32, tag="nmlx", name="nmlx")
            nc.vector.tensor_reduce(nmlx[:], ml[:], axis=X, op=A.max, negate=True)
            e4 = W.tile([B, 4], f32, tag="e4", name="e4")
            sse = W.tile([B, 1], f32, tag="sse", name="sse")
            nc.scalar.activation(e4[:], ml[:], F.Exp, bias=nmlx[:], accum_out=sse[:])
            lse = W.tile([B, 1], f32, tag="lse", name="lse")
            nc.scalar.activation(lse[:], sse[:], F.Ln)
            mls = W.tile([B, 1], f32, tag="mls", name="mls")
            ohml = W.tile([B, 4], f32, tag="ohml", name="ohml")
            nc.vector.scalar_tensor_tensor(ohml[:], oh[:], 1.0, ml[:], op0=A.mult,
                                           op1=A.mult, accum_out=mls[:])
            lgp2 = W.tile([B, 1], f32, tag="lgp2", name="lgp2")
            nc.vector.scalar_tensor_tensor(lgp2[:], mls[:], nmlx[:], lgp[:],
                                           op0=A.add, op1=A.add)
            nc.vector.tensor_scalar(lgp[:], lgp2[:], lse[:], None, op0=A.subtract)
            # token bits + counters
            nc.vector.tensor_reduce(cfg[:, t:t + 1], oh[:, 2:4], axis=X, op=A.add)
            ohodd = oh[:].rearrange("p (a b) -> p a b", a=2)[:, :, 1]
            nc.vector.tensor_reduce(cfg[:, N_ORB + t:N_ORB + t + 1], ohodd,
                                    axis=X, op=A.add)
            nc.vector.tensor_scalar(a_rem[:], a_rem[:], cfg[:, t:t + 1], None,
                                    op0=A.subtract)
            nc.vector.tensor_scalar(b_rem[:], b_rem[:],
                                    cfg[:, N_ORB + t:N_ORB + t + 1], None,
                                    op0=A.subtract)
            # next-token embedding (pos folded into embp)
            if t < N_ORB - 1:
                ohT_ps = transpose_pe(oh[:], B, 4)
                ohT = W.tile([4, B], f32, tag="ohT", name="ohT")
                nc.vector.tensor_copy(ohT[:], ohT_ps[:4, :B])
                embt = W.tile([4, D], f32, tag="embt", name="embt")
                nc.sync.dma_start(embt[:], din["embp"].ap()[:, t * D:(t + 1) * D])
                xe_ps = P.tile([B, D], f32, tag="ps", name="xe_ps")
                nc.tensor.matmul(xe_ps[:], ohT[:], embt[:])
                xnext = S.tile([B, D], f32, tag="x", name="x")
                nc.vector.tensor_copy(xnext[:], xe_ps[:])
                x = xnext

        nc.sync.dma_start(cfg_out.ap(), cfg[:])
        nc.sync.dma_start(lgp_out.ap(), lgp[:])

    nc.compile()
    return nc


def _prepare_inputs_per_core(inputs):
    """Host-side folds + per-core input maps."""
    f64 = np.float64
    state_emb = np.asarray(inputs["state_emb"], f64)
    pos_emb = np.asarray(inputs["pos_emb"], f64)
    ln1_w = np.asarray(inputs["ln1_w"], f64); ln1_b = np.asarray(inputs["ln1_b"], f64)
    in_w = np.asarray(inputs["in_proj_w"], f64); in_b = np.asarray(inputs["in_proj_b"], f64)
    out_w = np.asarray(inputs["out_proj_w"], f64); out_b = np.asarray(inputs["out_proj_b"], f64)
    ln2_w = np.asarray(inputs["ln2_w"], f64); ln2_b = np.asarray(inputs["ln2_b"], f64)
    ffn_w1 = np.asarray(inputs["ffn_w1"], f64); ffn_b1 = np.asarray(inputs["ffn_b1"], f64)
    ffn_w2 = np.asarray(inputs["ffn_w2"], f64); ffn_b2 = np.asarray(inputs["ffn_b2"], f64)
    fn_w = np.asarray(inputs["fn_w"], f64); fn_b = np.asarray(inputs["fn_b"], f64)
    head_w = np.asarray(inputs["head_w"], f64); head_b = np.asarray(inputs["head_b"], f64)

    com = {}
    com["wqkv"] = np.stack([(in_w[l] * ln1_w[l][None, :]).T for l in range(L)])
    bq = np.stack([in_b[l] + in_w[l] @ ln1_b[l] for l in range(L)])        # [L, 384]
    com["bqkvr"] = np.broadcast_to(bq[:, None, :], (L, B, 3 * D)).copy()
    com["wout"] = np.stack([out_w[l].T for l in range(L)])
    com["boutr"] = np.broadcast_to(out_b[:, None, :], (L, B, D)).copy()
    com["wff1"] = np.stack([(ffn_w1[l] * ln2_w[l][None, :]).T for l in range(L)])
    com["bff1"] = np.stack([(ffn_b1[l] + ffn_w1[l] @ ln2_b[l]).reshape(4, D).T
                            for l in range(L)])
    com["wff2"] = np.stack([ffn_w2[l].T.reshape(4, D, D) for l in range(L)])
    com["bff2r"] = np.broadcast_to(ffn_b2[:, None, :], (L, B, D)).copy()
    com["whead"] = (head_w * fn_w[None, :]).T
    bhead = head_b + head_w @ fn_b                                        # [4]
    com["bhmr"] = np.broadcast_to(bhead[None, :], (B, 4)).copy()
    # embedding rows with position t+1 folded in, one [4, D] block per step
    eb = np.stack([state_emb[:4] + pos_emb[tt + 1][None, :]
                   for tt in range(N_ORB - 1)])                           # [31, 4, D]
    com["embp"] = eb.transpose(1, 0, 2).reshape(4, (N_ORB - 1) * D)
    com["cA"] = np.broadcast_to(np.array([0, 0, 1, 1], f64), (B, 4)).copy()
    com["cB"] = np.broadcast_to(np.array([0, 1, 0, 1], f64), (B, 4)).copy()
    com["ident"] = np.eye(D)
    x0 = state_emb[BOS] + pos_emb[0]
    com = {k: np.ascontiguousarray(v, np.float32) for k, v in com.items()}

    # gumbel noise exactly as the reference draws it (reference only runs on
    # the CPU backend in this environment, so match CPU RNG lowering)
    import jax
    with jax.default_device(jax.devices("cpu")[0]):
        keys = jax.random.split(jax.random.key(42), N_ORB)
        g = np.stack([np.asarray(jax.random.gumbel(k, (NCORES * B, 4),
                                                   dtype=np.float32)) for k in keys])
    gfold = (g.astype(f64) + bhead[None, None, :]).astype(np.float32)

    maps = []
    for c in range(NCORES):
        m = dict(com)
        m["x0r"] = np.ascontiguousarray(
            np.broadcast_to(x0.astype(np.float32), (B, D)))
        gs = gfold[:, c * B:(c + 1) * B, :]
        m["gum"] = np.ascontiguousarray(gs.transpose(1, 0, 2).reshape(B, 4 * N_ORB))
        maps.append(m)
    return maps


def kernel(**inputs):
    n = int(np.asarray(inputs["n_samples"]))
    assert n == NCORES * B, f"kernel compiled for n_samples=1024, got {n}"
    if "prog" not in _PROG_CACHE:
        _PROG_CACHE["prog"] = _build_program()
    nc = _PROG_CACHE["prog"]
    maps = _prepare_inputs_per_core(inputs)
    from concourse.bass_utils import run_bass_kernel_spmd
    res = run_bass_kernel_spmd(nc, maps, list(range(NCORES)))
    cfgs = np.concatenate([res.results[c]["cfg"] for c in range(NCORES)], axis=0)
    lgps = np.concatenate([res.results[c]["lgp"][:, 0] for c in range(NCORES)],
                          axis=0)
    return cfgs.astype(np.float32), lgps.astype(np.float32)
